# revision 10
# baseline (speedup 1.0000x reference)
"""CIF (continuous integrate-and-fire) kernel for Trainium2, 8-core data parallel.

Formulation: the emitted frame for label k of batch row b is a weighted sum of
hidden rows:  out[b,k,:] = sum_t W[b,k,t] * hidden[b,t,:]  where the sparse
weights W follow from the sequential alpha-scan (fire decisions):
  - non-fire step t feeding label k:        W[k,t] = alpha[t]
  - fire step t_k (emits label k):          W[k,t_k] = 1 - integrate_{t_k-1}
  - fire step t_k also seeds label k+1:     W[k+1,t_k] = remainds_k
Contributions to labels that never fire (or >= max_label_len) are dropped.

The scalar scan over T (on the tiny [B,T] alphas) runs on host in exact fp32
program order, reproducing the reference's fire decisions bit-exactly; only the
w*h reduction runs in fp16 (fp32 PSUM accumulation) on device.

Device design (v2 — host-built slab weights):
  For each 128-step time chunk c, the labels it can touch span a ~24-wide
  window (union over the 8 rows sharing a program slot; SPMD runs one program
  on all cores). The host packs those weights (both diagonals folded in:
  w1 at seg_t, rem at seg_t+1) into dense per-chunk "slabs" whose columns map
  1:1 onto a legal matmul PSUM output region — [base, base+64) with base in
  {0, 64}, or the full bank [0, 128) when the window crosses partition 64.
  Per chunk the device then runs one accumulating matmul per touched bank:
      psum[bank][base : base+width, :] += slab.T @ hidden_chunk
  into pre-zeroed PSUM, then drains to fp16 and stores. No on-device weight
  construction at all -> DVE nearly idle, the PE matmul stream has no
  cross-engine dependencies beyond the DMAs, and the DMA stream is 13 large
  contiguous transfers.

Host also pre-transposes hidden into chunk-partition-major [128, NCH*H]
layout (tail chunk zero-padded), so every hidden DMA line is 16KB contiguous.

Sharding: pure data parallel over batch — each of the 8 cores handles B/8 rows.
"""

import sys

if "/opt/trn_rl_repo" not in sys.path:
    sys.path.insert(0, "/opt/trn_rl_repo")

from contextlib import ExitStack

import numpy as np

import concourse.bass as bass  # noqa: F401
import concourse.mybir as mybir
import concourse.tile as tile
from concourse import bacc
from concourse.bass_utils import run_bass_kernel_spmd

F32 = mybir.dt.float32
F16 = mybir.dt.float16

N_CORES = 8
NLAB = 256  # labels computed on device (= reference max_label_len)
CH = 128  # time-chunk size (contraction dim)
NCH = 16  # chunks (T=2000 padded to 2048)
NB = 2  # psum label banks of 128

_program_cache: dict = {}


def _host_scan(alphas: np.ndarray):
    """Replicate the reference integrate-and-fire scan in fp32, vectorized
    over batch. Returns per-step weights, target labels, and fire info."""
    alphas = np.ascontiguousarray(alphas, dtype=np.float32)
    B, T = alphas.shape
    one = np.float32(1.0)
    thr = np.float32(0.95)
    zero = np.float32(0.0)
    I = np.zeros(B, np.float32)
    nf = np.zeros(B, np.int32)
    w1 = np.empty((B, T), np.float32)
    seg = np.empty((B, T), np.int32)
    fires = np.zeros((B, T), bool)
    rem = np.empty((B, T), np.float32)
    for t in range(T):
        a = alphas[:, t]
        dist = one - I
        integ = I + a
        fire = integ > thr
        cur = np.where(fire, dist, a)
        w1[:, t] = cur
        rem[:, t] = a - cur  # remainder (only meaningful at fires)
        seg[:, t] = nf
        I = np.where(fire, integ - one, integ)
        nf = nf + fire
        fires[:, t] = fire
    # Drop contributions to labels that never fire.
    w1[seg >= nf[:, None]] = zero
    return w1, seg, fires, rem, nf


def _build_program(R: int, H: int, STRIDE: int, plan: tuple):
    """plan[r] = tuple of pieces (c, bank, base, off, width): one accumulating
    matmul psum[bank][base:base+width] += wt[:, r*STRIDE+off : +width].T @
    hidden_chunk_c. Derived from the actual input on host (union over the
    rows sharing each program slot); part of the compile cache key."""
    nc = bacc.Bacc("TRN2", target_bir_lowering=False, debug=False, num_devices=N_CORES)
    hidden = nc.dram_tensor("hidden", [R, CH, NCH * H], F16, kind="ExternalInput").ap()
    wt = nc.dram_tensor("wt", [CH, R * STRIDE], F16, kind="ExternalInput").ap()
    out = nc.dram_tensor("out", [R, NB, CH, H], F16, kind="ExternalOutput").ap()

    with tile.TileContext(nc) as tc, ExitStack() as ctx:
        wpool = ctx.enter_context(tc.tile_pool(name="wpool", bufs=1))
        hpool = ctx.enter_context(tc.tile_pool(name="hpool", bufs=1))
        opool = ctx.enter_context(tc.tile_pool(name="opool", bufs=1))
        pspool = ctx.enter_context(tc.tile_pool(name="pspool", bufs=1, space="PSUM"))

        # Load order (one sync-ring FIFO): row r's slab weights right before
        # row r's hidden pieces, so the PE is never gated on weights that
        # queued behind later rows' hidden data. Rows 0..R-2 load in halves
        # (big transfers keep the DMA at line rate); the final row tapers
        # (8+4+2+1+1 chunks) so the PE tail after the last byte is ~1 chunk.
        wtile = wpool.tile([CH, R * STRIDE], F16, name="wt", tag="wt")
        hrows = []
        for r in range(R):
            nc.sync.dma_start(
                wtile[:, r * STRIDE : (r + 1) * STRIDE],
                wt[:, r * STRIDE : (r + 1) * STRIDE],
            )
            ht = hpool.tile([CH, NCH * H], F16, name=f"h{r}", tag=f"h{r}")
            splits = (8, 4, 2, 1, 1) if r == R - 1 else (8, 8)
            c0 = 0
            for n in splits:
                nc.sync.dma_start(
                    ht[:, c0 * H : (c0 + n) * H], hidden[r, :, c0 * H : (c0 + n) * H]
                )
                c0 += n
            hrows.append(ht)

        # PSUM: 2 banks per row, all 8 banks used once. Pre-zero on DVE
        # (accumulating matmuls then never need start=True).
        ps = [
            [
                pspool.tile([CH, H], F32, name=f"ps{r}_{b}", tag=f"ps{r}_{b}")
                for b in range(NB)
            ]
            for r in range(R)
        ]
        for r in range(R):
            for b in range(NB):
                nc.vector.memset(ps[r][b][:], 0.0)

        for r in range(R):
            last = {}
            for i, (c, b, base, off, width) in enumerate(plan[r]):
                last[b] = i
            for i, (c, b, base, off, width) in enumerate(plan[r]):
                nc.tensor.matmul(
                    ps[r][b][base : base + width, :],
                    wtile[:, r * STRIDE + off : r * STRIDE + off + width],
                    hrows[r][:, c * H : (c + 1) * H],
                    start=False,
                    stop=(i == last[b]),
                    skip_group_check=True,
                )
            # Drain per bank (bank 0 on DVE, bank 1 on ACT — they run in
            # parallel, and bank 0's drain+store can start while bank 1 is
            # still accumulating), store per bank on the scalar ring (sync
            # ring keeps streaming hidden).
            ot = opool.tile([CH, NB * H], F16, name=f"ot{r}", tag=f"ot{r}")
            nc.vector.tensor_copy(ot[:, 0:H], ps[r][0][:])
            nc.scalar.copy(ot[:, H : 2 * H], ps[r][1][:])
            for b in range(NB):
                nc.scalar.dma_start(out[r, b], ot[:, b * H : (b + 1) * H])
    nc.compile()
    return nc


def _get_program(R: int, H: int, STRIDE: int, plan: tuple):
    key = (R, H, STRIDE, plan)
    if key not in _program_cache:
        _program_cache[key] = _build_program(R, H, STRIDE, plan)
    return _program_cache[key]


def _prepare_inputs(hidden: np.ndarray, alphas: np.ndarray):
    """Host scan + slab-weight packing + per-core device inputs."""
    B, T, H = hidden.shape
    R = -(-B // N_CORES)  # rows per core, padded
    B_pad = R * N_CORES

    w1, seg, fires, rem, nf = _host_scan(alphas)

    # Second diagonal: fire at step t (label seg_t) seeds label seg_t+1 with
    # weight rem_t, if that label is ever emitted.
    seg2 = seg + 1
    rem_ok = fires & (seg2 < nf[:, None]) & (seg2 < NLAB) & (rem != 0.0)
    w1_ok = w1 != 0.0

    # Label range per (slot, chunk): union over the rows sharing that program
    # slot across all cores (slot r handles rows {k*R + r}).
    INT_MAX = 1 << 30
    lab_lo = np.full((B_pad, NCH), INT_MAX, np.int64)
    lab_hi = np.full((B_pad, NCH), -1, np.int64)
    seg_m = np.where(w1_ok, seg, INT_MAX)
    seg_M = np.where(w1_ok, seg, -1)
    seg2_m = np.where(rem_ok, seg2, INT_MAX)
    seg2_M = np.where(rem_ok, seg2, -1)
    for c in range(NCH):
        t0, t1 = c * CH, min((c + 1) * CH, T)
        if t0 >= T:
            continue
        lab_lo[:B, c] = np.minimum(seg_m[:, t0:t1].min(1), seg2_m[:, t0:t1].min(1))
        lab_hi[:B, c] = np.maximum(seg_M[:, t0:t1].max(1), seg2_M[:, t0:t1].max(1))
    slot_lo = lab_lo.reshape(N_CORES, R, NCH).min(0)  # [R, NCH]
    slot_hi = lab_hi.reshape(N_CORES, R, NCH).max(0)

    # Build pieces: per (slot, chunk, touched bank) one slab whose columns map
    # onto a legal matmul PSUM region — [base, base+64) with base in {0, 64},
    # or [0, 128) if the bank-local window crosses partition 64.
    plan = []
    piece_of = {}  # (r, c, bank) -> (base, off)
    max_stride = 0
    for r in range(R):
        pieces = []
        off = 0
        for c in range(NCH):
            lo, hi = int(slot_lo[r, c]), int(slot_hi[r, c])
            if hi < 0:
                continue
            hi = min(hi, NLAB - 1)
            for bank in range(NB):
                a = max(lo, bank * 128) - bank * 128
                e = min(hi, bank * 128 + 127) - bank * 128
                if a > e:
                    continue
                b32 = (a // 32) * 32
                if b32 <= 64 and e <= b32 + 31:
                    base, width = b32, 32
                elif e < 64:
                    base, width = 0, 64
                elif a >= 64:
                    base, width = 64, 64
                else:
                    base, width = 0, 128
                pieces.append((c, bank, base, off, width))
                piece_of[(r, c, bank)] = (base, off)
                off += width
        plan.append(tuple(pieces))
        max_stride = max(max_stride, off)
    plan = tuple(plan)
    STRIDE = max_stride

    # Scatter both diagonals into the slab array [B, CH, STRIDE].
    wwin = np.zeros((B_pad, CH, STRIDE), np.float32)

    def scatter(mask, lab, val):
        bidx, tidx = np.nonzero(mask)
        labv = lab[bidx, tidx]
        slot = bidx % R
        c = tidx // CH
        p = tidx % CH
        bank = labv // 128
        base = np.empty(len(bidx), np.int64)
        off = np.empty(len(bidx), np.int64)
        for i in range(len(bidx)):
            base[i], off[i] = piece_of[(int(slot[i]), int(c[i]), int(bank[i]))]
        col = off + (labv - 128 * bank - base)
        np.add.at(wwin, (bidx, p, col), val[bidx, tidx])

    scatter(w1_ok, seg, w1)
    scatter(rem_ok, seg2, rem)
    wwin = wwin.astype(np.float16)

    # Hidden: chunk-partition-major fp16, tail chunk zero-padded to 128.
    hid = np.zeros((B_pad, CH, NCH, H), np.float16)
    nfull = T // CH
    hid[:B, :, :nfull, :] = (
        hidden[:, : nfull * CH].astype(np.float16).reshape(B, nfull, CH, H)
    ).transpose(0, 2, 1, 3)
    t_tail = nfull * CH
    if t_tail < T:
        hid[:B, : T - t_tail, nfull, :] = hidden[:, t_tail:].astype(np.float16)

    in_maps = []
    for k in range(N_CORES):
        rows = slice(k * R, (k + 1) * R)
        in_maps.append(
            {
                "hidden": hid[rows].reshape(R, CH, NCH * H),
                "wt": np.ascontiguousarray(
                    wwin[rows].transpose(1, 0, 2).reshape(CH, R * STRIDE)
                ),
            }
        )
    return in_maps, R, STRIDE, plan


def kernel(hidden: np.ndarray, alphas: np.ndarray, max_label_len) -> np.ndarray:
    hidden = np.asarray(hidden, dtype=np.float32)
    alphas = np.asarray(alphas, dtype=np.float32)
    L = int(max_label_len)
    B, T, H = hidden.shape

    in_maps, R, STRIDE, plan = _prepare_inputs(hidden, alphas)
    nc = _get_program(R, H, STRIDE, plan)
    res = run_bass_kernel_spmd(nc, in_maps, list(range(N_CORES)))
    # out[r] is [NB, 128, H] fp16: label = bank*128 + partition.
    full = np.concatenate(
        [
            np.asarray(res.results[k]["out"]).reshape(R, NB * CH, H)
            for k in range(N_CORES)
        ],
        axis=0,
    ).astype(np.float32)
    full = full[:B]  # drop padded rows

    if L <= NLAB:
        return np.ascontiguousarray(full[:, :L])
    pad = np.zeros((B, L - NLAB, H), np.float32)
    return np.concatenate([full, pad], axis=1)


# revision 14
# speedup vs baseline: 1.2302x; 1.2302x over previous
"""CIF (continuous integrate-and-fire) kernel for Trainium2, 8-core data parallel.

Formulation: the emitted frame for label k of batch row b is a weighted sum of
hidden rows:  out[b,k,:] = sum_t W[b,k,t] * hidden[b,t,:]  where the sparse
weights W follow from the sequential alpha-scan (fire decisions):
  - non-fire step t feeding label k:        W[k,t] = alpha[t]
  - fire step t_k (emits label k):          W[k,t_k] = 1 - integrate_{t_k-1}
  - fire step t_k also seeds label k+1:     W[k+1,t_k] = remainds_k
Contributions to labels that never fire (or >= max_label_len) are dropped.

The scalar scan over T (on the tiny [B,T] alphas) runs on host in exact fp32
program order, reproducing the reference's fire decisions bit-exactly; only the
w*h reduction runs in fp16 (fp32 PSUM accumulation) on device.

Device design (v2 — host-built slab weights):
  For each 128-step time chunk c, the labels it can touch span a ~24-wide
  window (union over the 8 rows sharing a program slot; SPMD runs one program
  on all cores). The host packs those weights (both diagonals folded in:
  w1 at seg_t, rem at seg_t+1) into dense per-chunk "slabs" whose columns map
  1:1 onto a legal matmul PSUM output region — [base, base+64) with base in
  {0, 64}, or the full bank [0, 128) when the window crosses partition 64.
  Per chunk the device then runs one accumulating matmul per touched bank:
      psum[bank][base : base+width, :] += slab.T @ hidden_chunk
  into pre-zeroed PSUM, then drains to fp16 and stores. No on-device weight
  construction at all -> DVE nearly idle, the PE matmul stream has no
  cross-engine dependencies beyond the DMAs, and the DMA stream is 13 large
  contiguous transfers.

Host also pre-transposes hidden into chunk-partition-major [128, NCH*H]
layout (tail chunk zero-padded), so every hidden DMA line is 16KB contiguous.

Sharding: pure data parallel over batch — each of the 8 cores handles B/8 rows.
"""

import sys

if "/opt/trn_rl_repo" not in sys.path:
    sys.path.insert(0, "/opt/trn_rl_repo")

from contextlib import ExitStack

import numpy as np

import concourse.bass as bass  # noqa: F401
import concourse.mybir as mybir
import concourse.tile as tile
from concourse import bacc
from concourse.bass_utils import run_bass_kernel_spmd

F32 = mybir.dt.float32
F16 = mybir.dt.float16

N_CORES = 8
NLAB = 256  # labels computed on device (= reference max_label_len)
CH = 128  # time-chunk size (contraction dim)
NCH = 16  # chunks (T=2000 padded to 2048)
NB = 2  # psum label banks of 128

_program_cache: dict = {}


def _host_scan(alphas: np.ndarray):
    """Replicate the reference integrate-and-fire scan in fp32, vectorized
    over batch. Returns per-step weights, target labels, and fire info."""
    alphas = np.ascontiguousarray(alphas, dtype=np.float32)
    B, T = alphas.shape
    one = np.float32(1.0)
    thr = np.float32(0.95)
    zero = np.float32(0.0)
    I = np.zeros(B, np.float32)
    nf = np.zeros(B, np.int32)
    w1 = np.empty((B, T), np.float32)
    seg = np.empty((B, T), np.int32)
    fires = np.zeros((B, T), bool)
    rem = np.empty((B, T), np.float32)
    for t in range(T):
        a = alphas[:, t]
        dist = one - I
        integ = I + a
        fire = integ > thr
        cur = np.where(fire, dist, a)
        w1[:, t] = cur
        rem[:, t] = a - cur  # remainder (only meaningful at fires)
        seg[:, t] = nf
        I = np.where(fire, integ - one, integ)
        nf = nf + fire
        fires[:, t] = fire
    # Drop contributions to labels that never fire.
    w1[seg >= nf[:, None]] = zero
    return w1, seg, fires, rem, nf


def _build_program(R: int, H: int, STRIDE: int, plan: tuple):
    """plan[r] = tuple of pieces (c, bank, base, off, width): one accumulating
    matmul psum[bank][base:base+width] += wt[:, r*STRIDE+off : +width].T @
    hidden_chunk_c. Derived from the actual input on host (union over the
    rows sharing each program slot); part of the compile cache key."""
    nc = bacc.Bacc("TRN2", target_bir_lowering=False, debug=False, num_devices=N_CORES)
    hidden = nc.dram_tensor("hidden", [R, CH, NCH * H], F16, kind="ExternalInput").ap()
    wt = nc.dram_tensor("wt", [CH, R * STRIDE], F16, kind="ExternalInput").ap()
    out = nc.dram_tensor("out", [R, CH, NB * H], F16, kind="ExternalOutput").ap()

    with tile.TileContext(nc) as tc, ExitStack() as ctx:
        wpool = ctx.enter_context(tc.tile_pool(name="wpool", bufs=1))
        hpool = ctx.enter_context(tc.tile_pool(name="hpool", bufs=1))
        opool = ctx.enter_context(tc.tile_pool(name="opool", bufs=1))
        pspool = ctx.enter_context(tc.tile_pool(name="pspool", bufs=1, space="PSUM"))

        # Load order (one sync-ring FIFO): row r's slab weights right before
        # row r's hidden pieces, so the PE is never gated on weights that
        # queued behind later rows' hidden data. Rows 0..R-2 load in halves
        # (big transfers keep the DMA at line rate); the final row tapers
        # (8+4+2+1+1 chunks) so the PE tail after the last byte is ~1 chunk.
        wtile = wpool.tile([CH, R * STRIDE], F16, name="wt", tag="wt")
        hrows = []
        for r in range(R):
            nc.sync.dma_start(
                wtile[:, r * STRIDE : (r + 1) * STRIDE],
                wt[:, r * STRIDE : (r + 1) * STRIDE],
            )
            ht = hpool.tile([CH, NCH * H], F16, name=f"h{r}", tag=f"h{r}")
            splits = (8, 4, 4) if r == R - 1 else (8, 8)
            c0 = 0
            for n in splits:
                nc.sync.dma_start(
                    ht[:, c0 * H : (c0 + n) * H], hidden[r, :, c0 * H : (c0 + n) * H]
                )
                c0 += n
            hrows.append(ht)

        # PSUM: 2 banks per row, all 8 banks used once. Pre-zero on DVE
        # (accumulating matmuls then never need start=True).
        ps = [
            [
                pspool.tile([CH, H], F32, name=f"ps{r}_{b}", tag=f"ps{r}_{b}")
                for b in range(NB)
            ]
            for r in range(R)
        ]
        for r in range(R):
            for b in range(NB):
                nc.vector.memset(ps[r][b][:], 0.0)

        for r in range(R):
            last = {}
            for i, (c, b, base, off, width) in enumerate(plan[r]):
                last[b] = i
            for i, (c, b, base, off, width) in enumerate(plan[r]):
                nc.tensor.matmul(
                    ps[r][b][base : base + width, :],
                    wtile[:, r * STRIDE + off : r * STRIDE + off + width],
                    hrows[r][:, c * H : (c + 1) * H],
                    start=False,
                    stop=(i == last[b]),
                    skip_group_check=True,
                )
            # Drain per bank (bank 0 on DVE, bank 1 on ACT — they run in
            # parallel, and bank 0's drain can start while bank 1 is still
            # accumulating), one store per row on the scalar ring (sync ring
            # keeps streaming hidden).
            ot = opool.tile([CH, NB * H], F16, name=f"ot{r}", tag=f"ot{r}")
            nc.vector.tensor_copy(ot[:, 0:H], ps[r][0][:])
            nc.scalar.copy(ot[:, H : 2 * H], ps[r][1][:])
            nc.scalar.dma_start(out[r], ot[:])
    nc.compile()
    return nc


def _get_program(R: int, H: int, STRIDE: int, plan: tuple):
    key = (R, H, STRIDE, plan)
    if key not in _program_cache:
        _program_cache[key] = _build_program(R, H, STRIDE, plan)
    return _program_cache[key]


def _prepare_inputs(hidden: np.ndarray, alphas: np.ndarray):
    """Host scan + slab-weight packing + per-core device inputs."""
    B, T, H = hidden.shape
    R = -(-B // N_CORES)  # rows per core, padded
    B_pad = R * N_CORES

    w1, seg, fires, rem, nf = _host_scan(alphas)

    # Second diagonal: fire at step t (label seg_t) seeds label seg_t+1 with
    # weight rem_t, if that label is ever emitted.
    seg2 = seg + 1
    rem_ok = fires & (seg2 < nf[:, None]) & (seg2 < NLAB) & (rem != 0.0)
    w1_ok = w1 != 0.0

    # Label range per (slot, chunk): union over the rows sharing that program
    # slot across all cores (slot r handles rows {k*R + r}).
    INT_MAX = 1 << 30
    lab_lo = np.full((B_pad, NCH), INT_MAX, np.int64)
    lab_hi = np.full((B_pad, NCH), -1, np.int64)
    seg_m = np.where(w1_ok, seg, INT_MAX)
    seg_M = np.where(w1_ok, seg, -1)
    seg2_m = np.where(rem_ok, seg2, INT_MAX)
    seg2_M = np.where(rem_ok, seg2, -1)
    for c in range(NCH):
        t0, t1 = c * CH, min((c + 1) * CH, T)
        if t0 >= T:
            continue
        lab_lo[:B, c] = np.minimum(seg_m[:, t0:t1].min(1), seg2_m[:, t0:t1].min(1))
        lab_hi[:B, c] = np.maximum(seg_M[:, t0:t1].max(1), seg2_M[:, t0:t1].max(1))
    slot_lo = lab_lo.reshape(N_CORES, R, NCH).min(0)  # [R, NCH]
    slot_hi = lab_hi.reshape(N_CORES, R, NCH).max(0)

    # Build pieces: per (slot, chunk, touched bank) one slab whose columns map
    # onto a legal matmul PSUM region — [base, base+64) with base in {0, 64},
    # or [0, 128) if the bank-local window crosses partition 64.
    plan = []
    piece_of = {}  # (r, c, bank) -> (base, off)
    max_stride = 0
    for r in range(R):
        pieces = []
        off = 0
        for c in range(NCH):
            lo, hi = int(slot_lo[r, c]), int(slot_hi[r, c])
            if hi < 0:
                continue
            hi = min(hi, NLAB - 1)
            for bank in range(NB):
                a = max(lo, bank * 128) - bank * 128
                e = min(hi, bank * 128 + 127) - bank * 128
                if a > e:
                    continue
                b32 = (a // 32) * 32
                if b32 <= 64 and e <= b32 + 31:
                    base, width = b32, 32
                elif e < 64:
                    base, width = 0, 64
                elif a >= 64:
                    base, width = 64, 64
                else:
                    base, width = 0, 128
                pieces.append((c, bank, base, off, width))
                piece_of[(r, c, bank)] = (base, off)
                off += width
        plan.append(tuple(pieces))
        max_stride = max(max_stride, off)
    plan = tuple(plan)
    STRIDE = max_stride

    # Scatter both diagonals into the slab array [B, CH, STRIDE].
    wwin = np.zeros((B_pad, CH, STRIDE), np.float32)

    def scatter(mask, lab, val):
        bidx, tidx = np.nonzero(mask)
        labv = lab[bidx, tidx]
        slot = bidx % R
        c = tidx // CH
        p = tidx % CH
        bank = labv // 128
        base = np.empty(len(bidx), np.int64)
        off = np.empty(len(bidx), np.int64)
        for i in range(len(bidx)):
            base[i], off[i] = piece_of[(int(slot[i]), int(c[i]), int(bank[i]))]
        col = off + (labv - 128 * bank - base)
        np.add.at(wwin, (bidx, p, col), val[bidx, tidx])

    scatter(w1_ok, seg, w1)
    scatter(rem_ok, seg2, rem)
    wwin = wwin.astype(np.float16)

    # Hidden: chunk-partition-major fp16, tail chunk zero-padded to 128.
    hid = np.zeros((B_pad, CH, NCH, H), np.float16)
    nfull = T // CH
    hid[:B, :, :nfull, :] = (
        hidden[:, : nfull * CH].astype(np.float16).reshape(B, nfull, CH, H)
    ).transpose(0, 2, 1, 3)
    t_tail = nfull * CH
    if t_tail < T:
        hid[:B, : T - t_tail, nfull, :] = hidden[:, t_tail:].astype(np.float16)

    in_maps = []
    for k in range(N_CORES):
        rows = slice(k * R, (k + 1) * R)
        in_maps.append(
            {
                "hidden": hid[rows].reshape(R, CH, NCH * H),
                "wt": np.ascontiguousarray(
                    wwin[rows].transpose(1, 0, 2).reshape(CH, R * STRIDE)
                ),
            }
        )
    return in_maps, R, STRIDE, plan


def kernel(hidden: np.ndarray, alphas: np.ndarray, max_label_len) -> np.ndarray:
    hidden = np.asarray(hidden, dtype=np.float32)
    alphas = np.asarray(alphas, dtype=np.float32)
    L = int(max_label_len)
    B, T, H = hidden.shape

    in_maps, R, STRIDE, plan = _prepare_inputs(hidden, alphas)
    nc = _get_program(R, H, STRIDE, plan)
    res = run_bass_kernel_spmd(nc, in_maps, list(range(N_CORES)))
    # out[r] is [128, 2*H] fp16: label = bank*128 + partition.
    full = np.concatenate(
        [
            np.asarray(res.results[k]["out"])
            .reshape(R, CH, NB, H)
            .transpose(0, 2, 1, 3)
            .reshape(R, NB * CH, H)
            for k in range(N_CORES)
        ],
        axis=0,
    ).astype(np.float32)
    full = full[:B]  # drop padded rows

    if L <= NLAB:
        return np.ascontiguousarray(full[:, :L])
    pad = np.zeros((B, L - NLAB, H), np.float32)
    return np.concatenate([full, pad], axis=1)
